# revision 20
# baseline (speedup 1.0000x reference)
"""Trainium2 Bass kernel for nn_CrossAttention_61890478735686.

Math per (batch n, unit u), with c = d = 256 channel indices, hw = 256:
    q = query[n] viewed [c, hw];  raw DRAM layout [hw, c] = q^T
    k = v = value[n] same.
    qW = q @ Wq[u]   [256, 64]
    kW = k @ Wk[u]   [256, 64]
    dot = qW @ kW^T  [256, 256];  attn = softmax(dot/16, axis=-1)
    vW = v @ Wv[u]   [256, 9]     (vW[d, m], d a channel index)
    out[c, m] = attn[c, :] @ vW   -> output[n, kh, kw, c, u], m = 3*kh+kw

Kernel dataflow (transposed so the softmax reduction axis d is the
partition/contraction axis of the final matmuls):
    qWT[qk, c] = Wq[u]^T @ q^T      (lhsT = Wq chunks, rhs = raw query)
    kWT[qk, c] = Wk[u]^T @ k^T
    dotT[d, c] = kWT^T-contraction over qk (lhsT = kWT cols, rhs = qWT)
    ET = exp(dotT / 16)             (ACT, PSUM -> SBUF fp16)
    vW_aug[d, m] = vW with a ones column m=9 -> emit row m=9 is the
                   softmax denominator S[c]
    emit (transposed): outT[c, m] = ET^T-contraction over d
        lhsT = ET chunk [d-chunk, c-chunk] (stationary),
        rhs  = vW_aug[d-chunk, 10] (moving, N=10)
    host: out = outT[:, :9] / outT[:, 9]

Perf structure (per core: 16 units x 16 batches):
    - all matmul operands fp16 (host-converted); PSUM accumulates fp32
    - vW computed once per batch for all 16 units (4 matmuls, N=144)
    - dot row-tiled: the two units of a pair occupy PE row groups 0-1 /
      2-3 (K=64 at partition base 0/64) and run concurrently on hardware
    - transposed emit keeps outputs on all 128 c-partitions, so the
      PSUM->SBUF staging copy is only 80 elem/lane per 4-unit group
      (vs 1024 for the [m, c] orientation)
    - kq PSUM->SBUF copies on DVE (gpsimd PSUM reads crash walrus codegen)
    - PSUM banks: qk 2x1 + dot 2x2 + vw 1 + outT 1 = 8

Sharding: tensor-parallel over units. Core i gets units 16i..16i+16 and
all 16 batches.
"""

import sys

if "/opt/trn_rl_repo" not in sys.path:
    sys.path.insert(0, "/opt/trn_rl_repo")

import numpy as np

import concourse.bass as bass
import concourse.tile as tile
from concourse import mybir
from concourse.bass_utils import run_bass_kernel_spmd

F32 = mybir.dt.float32
F16 = mybir.dt.float16

N_CORES = 8
NB = 16          # batches
UPC = 16         # units per core
C = 256          # channels
HW = 256         # h*w (contraction dim of the projections)
QK = 64          # qk_dim
M = 9            # kernel_len
MA = 10          # M + ones column
SCALE = 1.0 / 16.0

MMDT = F16
NPDT = np.float16


def split_multiwait_drains(nc):
    """This walrus build cannot codegen instructions carrying >1 sem wait
    (CoreV3GenImpl setupSyncWait: 'Too many sync wait commands').  Hoist
    all but the last wait into single-wait NOPs preceding the instruction
    on the same engine — semantically identical (the sequencer stalls on
    each in turn)."""
    for f in nc.m.functions:
        for bb in f.blocks:
            new_insts = []
            for inst in bb.instructions:
                si = getattr(inst, "sync_info", None)
                if si is not None and len(si.on_wait) > 1:
                    waits = list(si.on_wait)
                    for j, w in enumerate(waits[:-1]):
                        nop = mybir.InstNoOp(
                            name=f"{inst.name}-wsplit{j}",
                            engine=inst.engine,
                            ins=[],
                            outs=[],
                            sync_info=mybir.SyncInfo(on_wait=[w], on_update=[]),
                        )
                        new_insts.append(nop)
                    si.on_wait = [waits[-1]]
                new_insts.append(inst)
            bb.instructions = new_insts


def build_nc():
    nc = bass.Bass()

    q_d = nc.dram_tensor("query", [NB, HW, C], MMDT, kind="ExternalInput")
    v_d = nc.dram_tensor("value", [NB, HW, C], MMDT, kind="ExternalInput")
    wq_d = nc.dram_tensor("query_w", [UPC, HW, QK], MMDT, kind="ExternalInput")
    wk_d = nc.dram_tensor("key_w", [UPC, HW, QK], MMDT, kind="ExternalInput")
    wv_d = nc.dram_tensor("value_w", [UPC, HW, M], MMDT, kind="ExternalInput")
    ones_d = nc.dram_tensor("ones", [128, 1], MMDT, kind="ExternalInput")
    # outT per group of 4 units: [c-part p, unit u4, c-chunk cj, m]
    out_d = nc.dram_tensor("out", [NB, UPC // 4, 128, 4, 2, MA], F32,
                           kind="ExternalOutput")

    with tile.TileContext(nc) as tc:
        with (
            tc.tile_pool(name="persist", bufs=1) as persist,
            tc.tile_pool(name="kqp", bufs=3) as kqp,
            tc.tile_pool(name="etp", bufs=4) as etp,
            tc.tile_pool(name="vwp", bufs=2) as vwp,
            tc.tile_pool(name="outp", bufs=6) as outp,
            tc.tile_pool(name="ps_qk", bufs=2, space="PSUM") as ps_qk,
            tc.tile_pool(name="ps_dot", bufs=2, space="PSUM") as ps_dot,
            tc.tile_pool(name="ps_vw", bufs=1, space="PSUM") as ps_vw,
            tc.tile_pool(name="ps_out", bufs=1, space="PSUM") as ps_out,
        ):
            # ---- persistent inputs --------------------------------------
            # q_sb/v_sb: [p=128, n, k, c]; rows (k*128+p) of raw [hw, c]
            q_sb = persist.tile([128, NB, 2, C], MMDT)
            v_sb = persist.tile([128, NB, 2, C], MMDT)
            # wq/wk: [p, k, pair, (u2 qk)]  -> lhsT [128, 128] slices
            wq_sb = persist.tile([128, 2, UPC // 2, 2 * QK], MMDT)
            wk_sb = persist.tile([128, 2, UPC // 2, 2 * QK], MMDT)
            # wv: [p, k, u, m]
            wv_sb = persist.tile([128, 2, UPC, M], MMDT)
            ones_sb = persist.tile([128, 1], MMDT)
            # Input loading: few big DMAs (the ~650ns per-DMA DGE cost adds
            # up), spread over the DMA-capable queues so the first group's
            # operands arrive within a few us and compute ramps immediately.
            # first 2 batches of q/v lead their queues so the first group's
            # proj/dot can start as soon as the (queue-parallel) weight DMAs
            # land; remaining batches follow after the weights
            def dma_q(i):
                nc.sync.dma_start(
                    out=q_sb[:, 2 * i : 2 * i + 2],
                    in_=q_d[2 * i : 2 * i + 2].rearrange(
                        "n (k p) c -> p n k c", p=128
                    ),
                )

            def dma_v(i):
                nc.gpsimd.dma_start(
                    out=v_sb[:, 2 * i : 2 * i + 2],
                    in_=v_d[2 * i : 2 * i + 2].rearrange(
                        "n (k p) c -> p n k c", p=128
                    ),
                )

            nc.sync.dma_start(out=ones_sb[:], in_=ones_d[:])
            dma_q(0)
            dma_v(0)
            for k in range(2):
                nc.gpsimd.dma_start(
                    out=wv_sb[:, k],
                    in_=wv_d[:, 128 * k : 128 * (k + 1), :].rearrange("u p m -> p u m"),
                )
            # weights in two half-loads (pairs 0-3 first) so the first
            # group's lhsT tiles clear the shared DMA pool sooner
            for h in range(2):
                psl = slice(4 * h, 4 * h + 4)
                tsl = slice(8 * h, 8 * h + 8)
                nc.sync.dma_start(
                    out=wq_sb[:, 0, psl].rearrange("p pr (u q) -> p (pr u) q", u=2),
                    in_=wq_d[tsl, 0:128, :].rearrange("t p q -> p t q"),
                )
                nc.scalar.dma_start(
                    out=wq_sb[:, 1, psl].rearrange("p pr (u q) -> p (pr u) q", u=2),
                    in_=wq_d[tsl, 128:256, :].rearrange("t p q -> p t q"),
                )
                nc.sync.dma_start(
                    out=wk_sb[:, 0, psl].rearrange("p pr (u q) -> p (pr u) q", u=2),
                    in_=wk_d[tsl, 0:128, :].rearrange("t p q -> p t q"),
                )
                nc.scalar.dma_start(
                    out=wk_sb[:, 1, psl].rearrange("p pr (u q) -> p (pr u) q", u=2),
                    in_=wk_d[tsl, 128:256, :].rearrange("t p q -> p t q"),
                )
            for i in range(1, NB // 2):
                dma_q(i)
                dma_v(i)

            # ---- main loop (emit stage software-pipelined by 1 group) ----
            def emit_final(st):
                et_tiles, vw_aug, n, g = st
                # outT[c-part, u4, cj, m]: per (u4, cj) an accumulation
                # group over the two d-chunks dj; groups are sequential so
                # they may share the PSUM bank (zero region)
                psum_out = ps_out.tile([128, 4, 2, MA], F32, name="psum_out")
                for u4 in range(4):
                    sp, uu = divmod(u4, 2)
                    for cj in range(2):
                        for dj in range(2):
                            nc.tensor.matmul(
                                psum_out[:, u4, cj],
                                et_tiles[sp][:, uu, dj, 128 * cj : 128 * (cj + 1)],
                                vw_aug[:, dj, 4 * g + u4],
                                start=(dj == 0),
                                stop=(dj == 1),
                            )
                out_sb = outp.tile([128, 4, 2, MA], F32, name="out_sb")
                nc.vector.tensor_copy(out_sb[:], psum_out[:])
                nc.sync.dma_start(out=out_d[n, g], in_=out_sb[:])

            pending = None
            for n in range(NB):
                # vW for all 16 units of batch n: psum_vw[c-chunk j, u, m]
                psum_vw = ps_vw.tile([128, 2, UPC, M], F32, name="psum_vw")
                for j in range(2):
                    for k in range(2):
                        nc.tensor.matmul(
                            psum_vw[:, j],
                            v_sb[:, n, k, 128 * j : 128 * (j + 1)],
                            wv_sb[:, k],
                            start=(k == 0),
                            stop=(k == 1),
                        )
                # augmented [p, j, u, 10]: col 9 = 1.0 (softmax denom row)
                vw_aug = vwp.tile([128, 2, UPC, MA], MMDT, name="vw_aug")
                nc.vector.tensor_copy(vw_aug[:, :, :, 0:M], psum_vw[:])
                nc.vector.tensor_copy(
                    vw_aug[:, :, :, M:MA], ones_sb.to_broadcast([128, 2, UPC, 1])
                )

                for g in range(UPC // 4):  # group of 4 units
                    et_tiles = []
                    for sp in range(2):  # sub-pair of units
                        pr = 2 * g + sp
                        # qWT/kWT 2 units stacked: psum_qk[:,0]=q, [:,1]=k
                        psum_qk = ps_qk.tile([128, 2, C], F32, name="psum_qk")
                        for k in range(2):
                            nc.tensor.matmul(
                                psum_qk[:, 0],
                                wq_sb[:, k, pr],
                                q_sb[:, n, k],
                                start=(k == 0),
                                stop=(k == 1),
                            )
                        for k in range(2):
                            nc.tensor.matmul(
                                psum_qk[:, 1],
                                wk_sb[:, k, pr],
                                v_sb[:, n, k],
                                start=(k == 0),
                                stop=(k == 1),
                            )
                        kq_sb = kqp.tile([128, 2, C], MMDT, name="kq_sb")
                        nc.vector.tensor_copy(kq_sb[:], psum_qk[:])

                        # dotT: [d-chunk dj rows, c] per unit uu; the
                        # uu=0/uu=1 matmuls use PE row groups 0-1/2-3 (K=64
                        # at partition base 0/64) -> concurrent on hardware
                        psum_dot = ps_dot.tile(
                            [128, 2, 2, C], F32, name="psum_dot"
                        )  # [p, uu, dj, c]
                        for dj in range(2):
                            for uu in range(2):
                                nc.tensor.matmul(
                                    psum_dot[:, uu, dj],
                                    kq_sb[
                                        64 * uu : 64 * uu + 64,
                                        1,
                                        128 * dj : 128 * (dj + 1),
                                    ],
                                    kq_sb[64 * uu : 64 * uu + 64, 0],
                                    start=True,
                                    stop=True,
                                )
                        et_sb = etp.tile([128, 2, 2, C], MMDT, name="et_sb")
                        nc.scalar.activation(
                            out=et_sb[:],
                            in_=psum_dot[:],
                            func=mybir.ActivationFunctionType.Exp,
                            scale=SCALE,
                        )
                        et_tiles.append(et_sb)

                    if pending is not None:
                        emit_final(pending)
                    pending = (et_tiles, vw_aug, n, g)
            emit_final(pending)

    split_multiwait_drains(nc)
    return nc


_NC_CACHE = None


def _get_nc():
    global _NC_CACHE
    if _NC_CACHE is None:
        _NC_CACHE = build_nc()
    return _NC_CACHE


def make_in_maps(query, value, query_w, key_w, value_w):
    q = np.ascontiguousarray(query.reshape(NB, HW, C)).astype(NPDT)
    v = np.ascontiguousarray(value.reshape(NB, HW, C)).astype(NPDT)
    in_maps = []
    for i in range(N_CORES):
        sl = slice(UPC * i, UPC * (i + 1))
        in_maps.append(
            {
                "query": q,
                "value": v,
                "ones": np.ones((128, 1), dtype=NPDT),
                "query_w": np.ascontiguousarray(query_w[sl]).astype(NPDT),
                "key_w": np.ascontiguousarray(key_w[sl]).astype(NPDT),
                "value_w": np.ascontiguousarray(value_w[sl]).astype(NPDT),
            }
        )
    return in_maps


def core_out_to_norm(o):
    """[NB, 4, 128, 4, 2, MA] raw outT -> normalized [NB, UPC, M, C]."""
    o = np.asarray(o)
    # [n, g, p, u4, cj, m] -> [n, g, u4, m, cj, p] -> [n, u, m, c]
    t = o.transpose(0, 1, 3, 5, 4, 2).reshape(NB, UPC, MA, C)
    return t[:, :, :M, :] / t[:, :, M : M + 1, :]


def gather_output(core_outs):
    """core_outs: list of [NB, 4, 128, 4, 2, MA] -> full [NB, 3, 3, C, 128]."""
    full = np.empty((NB, 3, 3, C, 128), dtype=np.float32)
    for i, o in enumerate(core_outs):
        norm = core_out_to_norm(o)  # [n, u, 9, c]
        full[:, :, :, :, UPC * i : UPC * (i + 1)] = (
            norm.reshape(NB, UPC, 3, 3, C).transpose(0, 2, 3, 4, 1)
        )
    return full


def kernel(query, value, query_w, key_w, value_w):
    nc = _get_nc()
    in_maps = make_in_maps(query, value, query_w, key_w, value_w)
    res = run_bass_kernel_spmd(nc, in_maps, core_ids=list(range(N_CORES)))
    return gather_output([r["out"] for r in res.results])
